# revision 32
# baseline (speedup 1.0000x reference)
"""HLGAttention Trainium2 kernel (bf16 pipeline, software-pipelined).

Windowed multi-head attention over B=1024 independent windows of N=196
tokens, C=128 dims, 4 heads, with a dynamic (input-independent) relative
position bias. Windows are sharded 128-per-core across 8 NeuronCores.

Device layout notes (per window):
  - Input is pre-transposed + bf16-cast on host: xT [C=128, W, N=196] so
    projections run as out = W.T @ xT with C on partitions.
  - All matmul operands are bf16 (1 cyc/row vs fp32's 4); PSUM accumulates
    fp32. Measured end-to-end rel err ~6e-3 vs the 2e-2 gate.
  - S is computed transposed (keys on partitions) into one [98k, 4h, 512]
    PSUM tile (each head padded to a full 512-word bank so matmul outputs
    never cross bank bounds), so softmax's P lands ready to stream as the
    PV matmul's moving operand and exp is a single big activation call.
  - Bias is folded multiplicatively: P = exp(S) * exp(rpb) with exp(rpb) a
    bf16 SBUF constant; the multiply is one full-tile DVE tensor_tensor
    (contiguous bf16 -> 2x mode).  GPSIMD offload was tried and reverted:
    Pool shares the DVE's SBUF port and the contention costs more than the
    offload saves.
  - Denominators via ones-stationary matmuls into nd[:, 256:452], lane-
    aligned with the numerators at nd[:, 0:196]; 1/den is split between
    ACT (exp(-ln d): ln+exp live in one table set, no reload thrash) and
    DVE's bit-exact iterative divide, sized to balance the two queues.
  - proj writes y into nd's numerator region after osb-mul consumed it, so
    PSUM fits in 8 banks: st 4 + qk 1 + v 1 + nd(double-buffered) 2.
  - Three-deep software pipeline: iteration i issues ebmul(i-2),
    normalize(i-3), qkv(i), PV/den(i-2), proj(i-3), ST/exp(i).  Every op
    is >= 1 full iteration downstream of its producers, so no in-order
    engine queue head ever blocks on same-iteration work; measured PE/DVE/
    ACT occupancy 86-93%.
  - Output yT [C, W, N] bf16; host casts/transposes back and adds bproj.
"""

import sys

sys.path.insert(0, "/opt/trn_rl_repo")

import numpy as np
import ml_dtypes

import bass_rust
import concourse.bass as bass
import concourse.mybir as mybir
import concourse.tile as T
from concourse.bass_utils import run_bass_kernel_spmd

GS = 14
N = 196          # tokens per window
C = 128          # channels
H = 4            # heads
HD = 32          # head dim
B = 1024         # windows
NCORES = 8
W = B // NCORES  # windows per core
KC = 98          # keys chunk (2 chunks of 98)
G = 4            # windows per DMA group
FP = mybir.dt.float32
BF = mybir.dt.bfloat16
EPS = 1e-5
BF_NP = ml_dtypes.bfloat16

import os as _os
# feature knobs (bisect aids)
K_EXP1 = int(_os.environ.get("K_EXP1", "1"))          # 1: one exp over 4 banks; 0: per-head
K_POOL_EB = int(_os.environ.get("K_POOL_EB", "3"))    # heads of EB-mult on Pool (0: all DVE)
K_ACT_RECIP = int(_os.environ.get("K_ACT_RECIP", "170"))  # cols of 1/d via ACT ln+exp
K_EB_DMA = int(_os.environ.get("K_EB_DMA", "0"))      # 1: alternate ebmul Pool/DVE by window parity


class FixedTile(T.TileContext):
    """TileContext whose epilogue splits drain waits across NOPs.

    The stock epilogue attaches every proc's semaphore wait to a single
    Drain, which overflows this walrus's per-instruction sync-wait limit.
    """

    def _drain_and_barrier(self, tick_clock, wait_clock):
        ticks = list(tick_clock.global_clock)
        for i, tv in enumerate(ticks):
            if tv > 0:
                vec = [0] * len(ticks)
                vec[i] = tv
                nop = self.nc.sync.nop()
                wait_clock.add_sem_waits(
                    nop.ins, T.ScopedClock({None: bass_rust.VectorClock(vec)})
                )
        self.nc.sync.drain()
        self.nc.all_engine_barrier()
        assert self.sems is not None
        popped = self.nc._tile_sem_poison_stack.pop()
        assert popped is self._sem_poison
        # clear_and_free_semaphores emits EVENT_SEMAPHORE_RANGE_CLEAR, which
        # this walrus cannot encode; each run loads a fresh NEFF, so skip it.
        self.nc.all_engine_barrier()


def _split_waits(nc, cap=1):
    """Move excess per-instruction sem waits onto preceding same-engine NOPs.

    This walrus build rejects instructions carrying more than `cap` sync
    waits ("Too many sync wait commands"), while Tile freely attaches one
    wait per upstream proc.
    """
    total = 0
    for blk in nc.m.functions[0].blocks:
        insts = list(blk.instructions)
        out = []
        for inst in insts:
            si = inst.sync_info
            waits = list(si.on_wait) if si is not None else []
            if len(waits) > cap:
                extra, keep = waits[:-cap], waits[-cap:]
                for j in range(0, len(extra), cap):
                    nop = mybir.InstNoOp(
                        name=f"{inst.name}_xw{j}", engine=inst.engine,
                        sync_info=mybir.SyncInfo(on_wait=extra[j:j + cap], on_update=[]),
                        bass_nofuse=True)
                    out.append(nop)
                    total += 1
                inst.sync_info = mybir.SyncInfo(on_wait=keep, on_update=list(si.on_update))
            out.append(inst)
        blk.instructions = out
    return total


def _build(n_windows: int):
    nc = bass.Bass()
    xT = nc.dram_tensor("xT", [C, n_windows, N], BF, kind="ExternalInput")
    eb = nc.dram_tensor("eb", [KC, H, 2 * N], BF, kind="ExternalInput")
    wq = nc.dram_tensor("wq", [C, C], BF, kind="ExternalInput")
    wk = nc.dram_tensor("wk", [C, C], BF, kind="ExternalInput")
    wv = nc.dram_tensor("wv", [C, C], BF, kind="ExternalInput")
    wproj = nc.dram_tensor("wproj", [C, C], BF, kind="ExternalInput")
    ones = nc.dram_tensor("ones", [KC, HD], BF, kind="ExternalInput")
    yT = nc.dram_tensor("yT", [C, n_windows, N], BF, kind="ExternalOutput")

    from contextlib import ExitStack

    with FixedTile(nc) as tc, ExitStack() as es:
        cpool = es.enter_context(tc.tile_pool(name="consts", bufs=1))
        eb_sb = cpool.tile([KC, H, 2 * N], BF, tag="eb")
        wq_sb = cpool.tile([C, C], BF, tag="wq")
        wk_sb = cpool.tile([C, C], BF, tag="wk")
        wv_sb = cpool.tile([C, C], BF, tag="wv")
        wp_sb = cpool.tile([C, C], BF, tag="wp")
        on_sb = cpool.tile([KC, HD], BF, tag="ones")
        nc.sync.dma_start(eb_sb[:, :, :], eb[:, :, :])
        for sb, dr in [(wq_sb, wq), (wk_sb, wk), (wv_sb, wv),
                       (wp_sb, wproj), (on_sb, ones)]:
            nc.sync.dma_start(sb[:, :], dr[:, :])

        # SBUF pools
        xt_pool = es.enter_context(tc.tile_pool(name="xt", bufs=2))
        qkt_pool = es.enter_context(tc.tile_pool(name="qkt", bufs=2))
        v_pool = es.enter_context(tc.tile_pool(name="vsb", bufs=2))
        p_pool = es.enter_context(tc.tile_pool(name="psb", bufs=4))
        r_pool = es.enter_context(tc.tile_pool(name="rsb", bufs=2))
        l_pool = es.enter_context(tc.tile_pool(name="lnd", bufs=2))
        o_pool = es.enter_context(tc.tile_pool(name="osb", bufs=2))
        y_pool = es.enter_context(tc.tile_pool(name="ysb", bufs=2))

        # PSUM pools: st 4 banks, qk 1, v 1, nd 2 -> 8 banks.  proj reuses
        # nd's numerator region (after osb-mul has consumed it), so y needs
        # no bank of its own.
        ps_st = es.enter_context(tc.tile_pool(name="ps_st", bufs=1, space="PSUM"))
        ps_qk = es.enter_context(tc.tile_pool(name="ps_qk", bufs=1, space="PSUM"))
        ps_v = es.enter_context(tc.tile_pool(name="ps_v", bufs=1, space="PSUM"))
        ps_nd = es.enter_context(tc.tile_pool(name="ps_nd", bufs=2, space="PSUM"))

        PIPE = 2  # PV/proj of window i-PIPE issued alongside ST of window i

        xts = {}    # group idx -> xt tile
        v2s = {}    # pair idx -> v psum pair tile
        qkts = {}   # window -> qkt sbuf tile
        sts = {}    # window -> st psum tile
        psbs = {}   # window -> P sbuf tile
        vsbs = {}   # window -> v sbuf tile
        ysbs = {}   # group idx -> output staging tile

        def front_a(w):
            """input DMA + qkv projections + casts for window w."""
            g, j = divmod(w, G)
            if j == 0:
                xt = xt_pool.tile([C, G, N], BF, tag="xt", name=f"xt{g}")
                nc.sync.dma_start(xt[:, :, :], xT[:, g * G:(g + 1) * G, :])
                xts[g] = xt
            xt = xts[g]

            # qT | kT -> one psum bank, then SBUF bf16
            qk_ps = ps_qk.tile([C, 2 * N], FP, tag="qk", name=f"qk{w}")
            nc.tensor.matmul(qk_ps[:, 0:N], wq_sb[:, :], xt[:, j, :], start=True, stop=True)
            nc.tensor.matmul(qk_ps[:, N:2 * N], wk_sb[:, :], xt[:, j, :], start=True, stop=True)
            qkt = qkt_pool.tile([C, 2 * N], BF, tag="qkt", name=f"qkt{w}")
            nc.vector.tensor_copy(qkt[:, :], qk_ps[:, :])
            qkts[w] = qkt

            # v (keys on partitions): window PAIRS share one [98, 2, 256]
            # psum bank; one paired cast per two windows halves the DVE
            # dispatch + PSUM-read overhead.
            j2 = w % 2
            if j2 == 0:
                v2s[w // 2] = ps_v.tile([KC, 2, 2 * C], FP, tag="v", name=f"v{w}")
            v_ps = v2s[w // 2]
            for c in range(2):
                nc.tensor.matmul(v_ps[:, j2, c * C:(c + 1) * C],
                                 xt[:, j, c * KC:(c + 1) * KC],
                                 wv_sb[:, :], start=True, stop=True)
            if j2 == 1:
                vsb = v_pool.tile([KC, 2, 2 * C], BF, tag="vsb", name=f"vsb{w}")
                nc.vector.tensor_copy(vsb[:, :, :], v_ps[:, :, :])
                vsbs[w // 2] = vsb

        def front_b(w):
            """ST matmuls + exp for window w."""
            qkt = qkts[w]
            # ST: one [98k, 4h, 512] psum tile (4 banks; each head padded to a
            # full 512-word bank so matmul outputs never cross bank bounds);
            # chunk c covers keys 98c..98c+97
            st = ps_st.tile([KC, H, 512], FP, tag="st", name=f"st{w}")
            for h in range(H):
                for c in range(2):
                    nc.tensor.matmul(
                        st[:, h, c * N:(c + 1) * N],
                        qkt[32 * h:32 * h + 32, N + c * KC:N + (c + 1) * KC],
                        qkt[32 * h:32 * h + 32, 0:N],
                        start=True, stop=True, tile_position=(32 * h, 0),
                    )
            sts[w] = st

            # P = exp(ST) * EB   (one activation; bias-mult split DVE/Pool)
            psb = p_pool.tile([KC, H, 2 * N], BF, tag="psb", name=f"psb{w}")
            if K_EXP1:
                nc.scalar.activation(psb[:, :, :], st[:, :, 0:2 * N],
                                     mybir.ActivationFunctionType.Exp)
            else:
                for h in range(H):
                    nc.scalar.activation(psb[:, h, :], st[:, h, 0:2 * N],
                                         mybir.ActivationFunctionType.Exp)
            psbs[w] = psb

        def ebmul(w):
            """P *= exp(bias), issued 2 iterations after exp(w).

            K_EB_DMA=1: runs as a SWDGE compute-DMA (accum_op=mult) on the
            otherwise-idle DMA engines, freeing DVE entirely.  Fallback:
            DVE/Pool tensor_mul split."""
            psb = psbs[w]
            if K_EB_DMA and w % 2 == 0:
                # whole-window alternation: Pool handles even windows' bias
                # mult, DVE odd ones.  Keeping each op on ONE tensor avoids
                # the SBUF-port contention a same-tensor split provokes.
                nc.gpsimd.tensor_mul(psb[:, :, :], psb[:, :, :], eb_sb[:, :, :])
            else:
                nc.vector.tensor_mul(psb[:, :, :], psb[:, :, :], eb_sb[:, :, :])

        nds = {}    # window -> nd psum tile
        osbs = {}   # window -> normalized-out sbuf tile

        def backmm(w):
            """PV + denominator matmuls for window w (PE only)."""
            psb = psbs[w]
            vsb2 = vsbs[w // 2]
            if w % 2 == 1:
                vsbs.pop(w // 2)
            vsb = vsb2[:, w % 2, :]
            sts.pop(w, None)
            qkts.pop(w, None)

            # PV numerators + ones-matmul denominators, lane-aligned.
            # NOTE: each accumulation group's matmuls must be consecutive --
            # start=True clears has_written for the WHOLE bank.
            nd = ps_nd.tile([C, 512], FP, tag="nd", name=f"nd{w}")
            for h in range(H):
                for c in range(2):
                    psl = psb[:, h, c * N:(c + 1) * N]
                    nc.tensor.matmul(nd[32 * h:32 * h + 32, 0:N],
                                     vsb[:, c * C + 32 * h: c * C + 32 * h + 32],
                                     psl, start=(c == 0), stop=(c == 1),
                                     tile_position=(0, 32 * h))
                for c in range(2):
                    psl = psb[:, h, c * N:(c + 1) * N]
                    nc.tensor.matmul(nd[32 * h:32 * h + 32, 256:256 + N],
                                     on_sb[:, :], psl, start=(c == 0), stop=(c == 1),
                                     tile_position=(0, 32 * h))
            nds[w] = nd

        def norm(w):
            """1/den + normalize for window w (ACT + DVE; den is a full
            iteration old so every op is ready when dequeued).

            ACT computes exp(-ln d) for the first RA cols (ln+exp share the
            natural_log_exp_and_others table set, so no table thrash); DVE's
            bit-exact iterative divide covers the rest."""
            psbs.pop(w, None)
            nd = nds[w]
            RA = K_ACT_RECIP
            rsb = r_pool.tile([C, N], FP, tag="rsb", name=f"rsb{w}")
            if RA:
                lnd = l_pool.tile([C, RA], FP, tag="lnd", name=f"lnd{w}")
                nc.scalar.activation(lnd[:, :], nd[:, 256:256 + RA],
                                     mybir.ActivationFunctionType.Ln)
                nc.scalar.activation(rsb[:, 0:RA], lnd[:, :],
                                     mybir.ActivationFunctionType.Exp, scale=-1.0)
            if RA < N:
                nc.vector.reciprocal(rsb[:, RA:N], nd[:, 256 + RA:256 + N])
            osb = o_pool.tile([C, N], BF, tag="osb", name=f"osb{w}")
            nc.vector.tensor_mul(osb[:, :], nd[:, 0:N], rsb[:, :])
            osbs[w] = osb

        def fin(w):
            """proj + output staging + DMA for window w.  proj overwrites
            nd's numerator region -- osb-mul already consumed it."""
            g, j = divmod(w, G)
            nd = nds.pop(w)
            osb = osbs.pop(w)

            # yT = wproj.T @ out_normT (bias added on host)
            nc.tensor.matmul(nd[:, 0:N], wp_sb[:, :], osb[:, :], start=True, stop=True)

            if j == 0:
                ysbs[g] = y_pool.tile([C, G, N], BF, tag="ysb", name=f"ysb{g}")
            nc.vector.tensor_copy(ysbs[g][:, j, :], nd[:, 0:N])
            if j == G - 1:
                ysb = ysbs.pop(g)
                nc.sync.dma_start(yT[:, g * G:(g + 1) * G, :], ysb[:, :, :])

        # Software pipeline, steady-state iteration i:
        #   DVE: ebmul(i-2), recip/osb(i-3), qkt/vsb casts(i)
        #   ACT: ln/exp2(i-3), ysb-copy(i-3), exp(i)
        #   PE : qk/v(i), PV/den(i-2), proj(i-3), st(i)
        # Every op is >= 1 iteration downstream of its producers, so no
        # engine queue head ever blocks on same-iteration work.
        D2, D3 = PIPE, PIPE + 1
        for i in range(n_windows + D3):
            if 0 <= i - D2 < n_windows:
                ebmul(i - D2)
            if 0 <= i - D3 < n_windows:
                norm(i - D3)
            if i < n_windows:
                front_a(i)
            if 0 <= i - D2 < n_windows:
                backmm(i - D2)
            if 0 <= i - D3 < n_windows:
                fin(i - D3)
            if i < n_windows:
                front_b(i)

    _split_waits(nc)
    return nc


def _host_bias(pp_w, pp_b, ln1_g, ln1_b, l1_w, l1_b, ln2_g, ln2_b, l2_w, l2_b,
               ln3_g, ln3_b, l3_w, l3_b):
    """Replicates the reference's tiny position-bias MLP in numpy fp32."""
    p = np.arange(1 - GS, GS)
    bb = np.stack(np.meshgrid(p, p, indexing="ij")).reshape(2, -1).T.astype(np.float32)

    def ln(x, g, b):
        mu = x.mean(-1, keepdims=True)
        var = ((x - mu) ** 2).mean(-1, keepdims=True)
        return (x - mu) / np.sqrt(var + EPS) * g + b

    pos = bb @ pp_w + pp_b
    pos = np.maximum(ln(pos, ln1_g, ln1_b), 0) @ l1_w + l1_b
    pos = np.maximum(ln(pos, ln2_g, ln2_b), 0) @ l2_w + l2_b
    pos = np.maximum(ln(pos, ln3_g, ln3_b), 0) @ l3_w + l3_b   # [729, H]

    ch = np.arange(GS)
    coords = np.stack(np.meshgrid(ch, ch, indexing="ij")).reshape(2, -1)
    rel = coords[:, :, None] - coords[:, None, :]
    rel = rel.transpose(1, 2, 0) + (GS - 1)
    idx = rel[..., 0] * (2 * GS - 1) + rel[..., 1]               # [N, N]
    return pos[idx]                                              # [N, N, H] = bias[q,k,h]


_NC_CACHE = {}


def kernel(**inputs):
    x = np.asarray(inputs["x"], dtype=np.float32)
    scale = np.float32(HD) ** -0.5

    rpb = _host_bias(*[np.asarray(inputs[k], dtype=np.float32) for k in
                       ("pp_w", "pp_b", "ln1_g", "ln1_b", "l1_w", "l1_b",
                        "ln2_g", "ln2_b", "l2_w", "l2_b",
                        "ln3_g", "ln3_b", "l3_w", "l3_b")])
    # EB[r, (h, c, q)] = exp(bias[q, 98c+r, h]) matching ST tile layout
    ebt = np.exp(rpb.transpose(2, 1, 0))            # [H, k, q]
    ebm = np.empty((KC, H, 2, N), dtype=np.float32)
    for c in range(2):
        ebm[:, :, c, :] = ebt.transpose(1, 0, 2)[c * KC:(c + 1) * KC]
    ebm = ebm.reshape(KC, H, 2 * N)

    wkv = np.asarray(inputs["wkv"], dtype=np.float32)
    consts = {
        "eb": np.ascontiguousarray(ebm.astype(BF_NP)),
        "wq": np.ascontiguousarray((np.asarray(inputs["wq"], np.float32) * scale).astype(BF_NP)),
        "wk": np.ascontiguousarray(wkv[:, :C].astype(BF_NP)),
        "wv": np.ascontiguousarray(wkv[:, C:].astype(BF_NP)),
        "wproj": np.ascontiguousarray(np.asarray(inputs["wproj"], np.float32).astype(BF_NP)),
        "ones": np.ones((KC, HD), dtype=BF_NP),
    }
    bproj = np.asarray(inputs["bproj"], dtype=np.float32)

    # [B, N, C] -> per-core [C, W, N] bf16
    xt_all = x.transpose(2, 0, 1).astype(BF_NP)      # [C, B, N]

    if W not in _NC_CACHE:
        _NC_CACHE[W] = _build(W)
    nc = _NC_CACHE[W]

    in_maps = []
    for core in range(NCORES):
        m = dict(consts)
        m["xT"] = np.ascontiguousarray(xt_all[:, core * W:(core + 1) * W, :])
        in_maps.append(m)

    import os
    trace = bool(os.environ.get("BASS_KERNEL_TRACE"))
    res = run_bass_kernel_spmd(nc, in_maps, core_ids=list(range(NCORES)),
                               trace=trace)
    global LAST_RESULT
    LAST_RESULT = res

    out = np.empty((B, N, C), dtype=np.float32)
    for core in range(NCORES):
        yt = res.results[core]["yT"]                 # [C, W, N] bf16
        out[core * W:(core + 1) * W] = yt.astype(np.float32).transpose(1, 2, 0)
    out += bproj[None, None, :]
    return out


LAST_RESULT = None


# revision 33
# speedup vs baseline: 1.0495x; 1.0495x over previous
"""HLGAttention Trainium2 kernel (bf16 pipeline, software-pipelined).

Windowed multi-head attention over B=1024 independent windows of N=196
tokens, C=128 dims, 4 heads, with a dynamic (input-independent) relative
position bias. Windows are sharded 128-per-core across 8 NeuronCores.

Device layout notes (per window):
  - Input is pre-transposed + bf16-cast on host: xT [C=128, W, N=196] so
    projections run as out = W.T @ xT with C on partitions.
  - All matmul operands are bf16 (1 cyc/row vs fp32's 4); PSUM accumulates
    fp32. Measured end-to-end rel err ~6e-3 vs the 2e-2 gate.
  - S is computed transposed (keys on partitions) into one [98k, 4h, 512]
    PSUM tile (each head padded to a full 512-word bank so matmul outputs
    never cross bank bounds), so softmax's P lands ready to stream as the
    PV matmul's moving operand and exp is a single big activation call.
  - Bias is folded multiplicatively: P = exp(S) * exp(rpb) with exp(rpb) a
    bf16 SBUF constant; the multiply is one full-tile DVE tensor_tensor
    (contiguous bf16 -> 2x mode).  GPSIMD offload was tried and reverted:
    Pool shares the DVE's SBUF port and the contention costs more than the
    offload saves.
  - Denominators via ones-stationary matmuls into nd[:, 256:452], lane-
    aligned with the numerators at nd[:, 0:196]; 1/den is split between
    ACT (exp(-ln d): ln+exp live in one table set, no reload thrash) and
    DVE's bit-exact iterative divide, sized to balance the two queues.
  - proj writes y into nd's numerator region after osb-mul consumed it, so
    PSUM fits in 8 banks: st 4 + qk 1 + v 1 + nd(double-buffered) 2.
  - Three-deep software pipeline: iteration i issues ebmul(i-2),
    normalize(i-3), qkv(i), PV/den(i-2), proj(i-3), ST/exp(i).  Every op
    is >= 1 full iteration downstream of its producers, so no in-order
    engine queue head ever blocks on same-iteration work; measured PE/DVE/
    ACT occupancy 86-93%.
  - Output yT [C, W, N] bf16; host casts/transposes back and adds bproj.
"""

import sys

sys.path.insert(0, "/opt/trn_rl_repo")

import numpy as np
import ml_dtypes

import bass_rust
import concourse.bass as bass
import concourse.mybir as mybir
import concourse.tile as T
from concourse.bass_utils import run_bass_kernel_spmd

GS = 14
N = 196          # tokens per window
C = 128          # channels
H = 4            # heads
HD = 32          # head dim
B = 1024         # windows
NCORES = 8
W = B // NCORES  # windows per core
KC = 98          # keys chunk (2 chunks of 98)
G = 4            # windows per DMA group
FP = mybir.dt.float32
BF = mybir.dt.bfloat16
EPS = 1e-5
BF_NP = ml_dtypes.bfloat16

import os as _os
# feature knobs (bisect aids)
K_EXP1 = int(_os.environ.get("K_EXP1", "1"))          # 1: one exp over 4 banks; 0: per-head
K_POOL_EB = int(_os.environ.get("K_POOL_EB", "3"))    # heads of EB-mult on Pool (0: all DVE)
K_ACT_RECIP = int(_os.environ.get("K_ACT_RECIP", "170"))  # cols of 1/d via ACT ln+exp
K_EB_DMA = int(_os.environ.get("K_EB_DMA", "0"))      # 1: alternate ebmul Pool/DVE by window parity


class FixedTile(T.TileContext):
    """TileContext whose epilogue splits drain waits across NOPs.

    The stock epilogue attaches every proc's semaphore wait to a single
    Drain, which overflows this walrus's per-instruction sync-wait limit.
    """

    def _drain_and_barrier(self, tick_clock, wait_clock):
        ticks = list(tick_clock.global_clock)
        for i, tv in enumerate(ticks):
            if tv > 0:
                vec = [0] * len(ticks)
                vec[i] = tv
                nop = self.nc.sync.nop()
                wait_clock.add_sem_waits(
                    nop.ins, T.ScopedClock({None: bass_rust.VectorClock(vec)})
                )
        self.nc.sync.drain()
        self.nc.all_engine_barrier()
        assert self.sems is not None
        popped = self.nc._tile_sem_poison_stack.pop()
        assert popped is self._sem_poison
        # clear_and_free_semaphores emits EVENT_SEMAPHORE_RANGE_CLEAR, which
        # this walrus cannot encode; each run loads a fresh NEFF, so skip it.
        self.nc.all_engine_barrier()


def _split_waits(nc, cap=1):
    """Move excess per-instruction sem waits onto preceding same-engine NOPs.

    This walrus build rejects instructions carrying more than `cap` sync
    waits ("Too many sync wait commands"), while Tile freely attaches one
    wait per upstream proc.
    """
    total = 0
    for blk in nc.m.functions[0].blocks:
        insts = list(blk.instructions)
        out = []
        for inst in insts:
            si = inst.sync_info
            waits = list(si.on_wait) if si is not None else []
            if len(waits) > cap:
                extra, keep = waits[:-cap], waits[-cap:]
                for j in range(0, len(extra), cap):
                    nop = mybir.InstNoOp(
                        name=f"{inst.name}_xw{j}", engine=inst.engine,
                        sync_info=mybir.SyncInfo(on_wait=extra[j:j + cap], on_update=[]),
                        bass_nofuse=True)
                    out.append(nop)
                    total += 1
                inst.sync_info = mybir.SyncInfo(on_wait=keep, on_update=list(si.on_update))
            out.append(inst)
        blk.instructions = out
    return total


def _build(n_windows: int):
    nc = bass.Bass()
    xT = nc.dram_tensor("xT", [C, n_windows, N], BF, kind="ExternalInput")
    eb = nc.dram_tensor("eb", [KC, H, 2 * N], BF, kind="ExternalInput")
    wq = nc.dram_tensor("wq", [C, C], BF, kind="ExternalInput")
    wk = nc.dram_tensor("wk", [C, C], BF, kind="ExternalInput")
    wv = nc.dram_tensor("wv", [C, C], BF, kind="ExternalInput")
    wproj = nc.dram_tensor("wproj", [C, C], BF, kind="ExternalInput")
    ones = nc.dram_tensor("ones", [KC, HD], BF, kind="ExternalInput")
    yT = nc.dram_tensor("yT", [C, n_windows, N], BF, kind="ExternalOutput")

    from contextlib import ExitStack

    with FixedTile(nc) as tc, ExitStack() as es:
        cpool = es.enter_context(tc.tile_pool(name="consts", bufs=1))
        eb_sb = cpool.tile([KC, H, 2 * N], BF, tag="eb")
        wq_sb = cpool.tile([C, C], BF, tag="wq")
        wk_sb = cpool.tile([C, C], BF, tag="wk")
        wv_sb = cpool.tile([C, C], BF, tag="wv")
        wp_sb = cpool.tile([C, C], BF, tag="wp")
        on_sb = cpool.tile([KC, HD], BF, tag="ones")
        nc.sync.dma_start(eb_sb[:, :, :], eb[:, :, :])
        for sb, dr in [(wq_sb, wq), (wk_sb, wk), (wv_sb, wv),
                       (wp_sb, wproj), (on_sb, ones)]:
            nc.sync.dma_start(sb[:, :], dr[:, :])

        # SBUF pools
        xt_pool = es.enter_context(tc.tile_pool(name="xt", bufs=2))
        qkt_pool = es.enter_context(tc.tile_pool(name="qkt", bufs=2))
        v_pool = es.enter_context(tc.tile_pool(name="vsb", bufs=4))
        p_pool = es.enter_context(tc.tile_pool(name="psb", bufs=4))
        r_pool = es.enter_context(tc.tile_pool(name="rsb", bufs=2))
        l_pool = es.enter_context(tc.tile_pool(name="lnd", bufs=2))
        o_pool = es.enter_context(tc.tile_pool(name="osb", bufs=2))
        y_pool = es.enter_context(tc.tile_pool(name="ysb", bufs=2))

        # PSUM pools: st 4 banks, qk 1, v 1, nd 2 -> 8 banks.  proj reuses
        # nd's numerator region (after osb-mul has consumed it), so y needs
        # no bank of its own.
        ps_st = es.enter_context(tc.tile_pool(name="ps_st", bufs=1, space="PSUM"))
        ps_qk = es.enter_context(tc.tile_pool(name="ps_qk", bufs=1, space="PSUM"))
        ps_v = es.enter_context(tc.tile_pool(name="ps_v", bufs=1, space="PSUM"))
        ps_nd = es.enter_context(tc.tile_pool(name="ps_nd", bufs=2, space="PSUM"))

        PIPE = 2  # PV/proj of window i-PIPE issued alongside ST of window i

        xts = {}    # group idx -> xt tile
        qkts = {}   # window -> qkt sbuf tile
        sts = {}    # window -> st psum tile
        psbs = {}   # window -> P sbuf tile
        vsbs = {}   # window -> v sbuf tile
        ysbs = {}   # group idx -> output staging tile

        def front_a(w):
            """input DMA + qkv projections + casts for window w."""
            g, j = divmod(w, G)
            if j == 0:
                xt = xt_pool.tile([C, G, N], BF, tag="xt", name=f"xt{g}")
                nc.sync.dma_start(xt[:, :, :], xT[:, g * G:(g + 1) * G, :])
                xts[g] = xt
            xt = xts[g]

            # qT | kT -> one psum bank, then SBUF bf16
            qk_ps = ps_qk.tile([C, 2 * N], FP, tag="qk", name=f"qk{w}")
            nc.tensor.matmul(qk_ps[:, 0:N], wq_sb[:, :], xt[:, j, :], start=True, stop=True)
            nc.tensor.matmul(qk_ps[:, N:2 * N], wk_sb[:, :], xt[:, j, :], start=True, stop=True)
            qkt = qkt_pool.tile([C, 2 * N], BF, tag="qkt", name=f"qkt{w}")
            nc.vector.tensor_copy(qkt[:, :], qk_ps[:, :])
            qkts[w] = qkt

            # v (keys on partitions), both token chunks -> one psum bank
            v_ps = ps_v.tile([KC, 2 * C], FP, tag="v", name=f"v{w}")
            for c in range(2):
                nc.tensor.matmul(v_ps[:, c * C:(c + 1) * C], xt[:, j, c * KC:(c + 1) * KC],
                                 wv_sb[:, :], start=True, stop=True)
            vsb = v_pool.tile([KC, 2 * C], BF, tag="vsb", name=f"vsb{w}")
            nc.vector.tensor_copy(vsb[:, :], v_ps[:, :])
            vsbs[w] = vsb

        def front_b(w):
            """ST matmuls + exp for window w."""
            qkt = qkts[w]
            # ST: one [98k, 4h, 512] psum tile (4 banks; each head padded to a
            # full 512-word bank so matmul outputs never cross bank bounds);
            # chunk c covers keys 98c..98c+97
            st = ps_st.tile([KC, H, 512], FP, tag="st", name=f"st{w}")
            for h in range(H):
                for c in range(2):
                    nc.tensor.matmul(
                        st[:, h, c * N:(c + 1) * N],
                        qkt[32 * h:32 * h + 32, N + c * KC:N + (c + 1) * KC],
                        qkt[32 * h:32 * h + 32, 0:N],
                        start=True, stop=True, tile_position=(32 * h, 0),
                    )
            sts[w] = st

            # P = exp(ST) * EB   (one activation; bias-mult split DVE/Pool)
            psb = p_pool.tile([KC, H, 2 * N], BF, tag="psb", name=f"psb{w}")
            if K_EXP1:
                nc.scalar.activation(psb[:, :, :], st[:, :, 0:2 * N],
                                     mybir.ActivationFunctionType.Exp)
            else:
                for h in range(H):
                    nc.scalar.activation(psb[:, h, :], st[:, h, 0:2 * N],
                                         mybir.ActivationFunctionType.Exp)
            psbs[w] = psb

        def ebmul(w):
            """P *= exp(bias), issued 2 iterations after exp(w).

            K_EB_DMA=1: runs as a SWDGE compute-DMA (accum_op=mult) on the
            otherwise-idle DMA engines, freeing DVE entirely.  Fallback:
            DVE/Pool tensor_mul split."""
            psb = psbs[w]
            if K_EB_DMA and w % 2 == 0:
                # whole-window alternation: Pool handles even windows' bias
                # mult, DVE odd ones.  Keeping each op on ONE tensor avoids
                # the SBUF-port contention a same-tensor split provokes.
                nc.gpsimd.tensor_mul(psb[:, :, :], psb[:, :, :], eb_sb[:, :, :])
            else:
                nc.vector.tensor_mul(psb[:, :, :], psb[:, :, :], eb_sb[:, :, :])

        nds = {}    # window -> nd psum tile
        osbs = {}   # window -> normalized-out sbuf tile

        def backmm(w):
            """PV + denominator matmuls for window w (PE only)."""
            psb, vsb = psbs[w], vsbs.pop(w)
            sts.pop(w, None)
            qkts.pop(w, None)

            # PV numerators + ones-matmul denominators, lane-aligned.
            # NOTE: each accumulation group's matmuls must be consecutive --
            # start=True clears has_written for the WHOLE bank.
            nd = ps_nd.tile([C, 512], FP, tag="nd", name=f"nd{w}")
            for h in range(H):
                for c in range(2):
                    psl = psb[:, h, c * N:(c + 1) * N]
                    nc.tensor.matmul(nd[32 * h:32 * h + 32, 0:N],
                                     vsb[:, c * C + 32 * h: c * C + 32 * h + 32],
                                     psl, start=(c == 0), stop=(c == 1),
                                     tile_position=(0, 32 * h))
                for c in range(2):
                    psl = psb[:, h, c * N:(c + 1) * N]
                    nc.tensor.matmul(nd[32 * h:32 * h + 32, 256:256 + N],
                                     on_sb[:, :], psl, start=(c == 0), stop=(c == 1),
                                     tile_position=(0, 32 * h))
            nds[w] = nd

        def norm(w):
            """1/den + normalize for window w (ACT + DVE; den is a full
            iteration old so every op is ready when dequeued).

            ACT computes exp(-ln d) for the first RA cols (ln+exp share the
            natural_log_exp_and_others table set, so no table thrash); DVE's
            bit-exact iterative divide covers the rest."""
            psbs.pop(w, None)
            nd = nds[w]
            RA = K_ACT_RECIP
            rsb = r_pool.tile([C, N], FP, tag="rsb", name=f"rsb{w}")
            if RA:
                lnd = l_pool.tile([C, RA], FP, tag="lnd", name=f"lnd{w}")
                nc.scalar.activation(lnd[:, :], nd[:, 256:256 + RA],
                                     mybir.ActivationFunctionType.Ln)
                nc.scalar.activation(rsb[:, 0:RA], lnd[:, :],
                                     mybir.ActivationFunctionType.Exp, scale=-1.0)
            if RA < N:
                nc.vector.reciprocal(rsb[:, RA:N], nd[:, 256 + RA:256 + N])
            osb = o_pool.tile([C, N], BF, tag="osb", name=f"osb{w}")
            nc.vector.tensor_mul(osb[:, :], nd[:, 0:N], rsb[:, :])
            osbs[w] = osb

        def fin(w):
            """proj + output staging + DMA for window w.  proj overwrites
            nd's numerator region -- osb-mul already consumed it."""
            g, j = divmod(w, G)
            nd = nds.pop(w)
            osb = osbs.pop(w)

            # yT = wproj.T @ out_normT (bias added on host)
            nc.tensor.matmul(nd[:, 0:N], wp_sb[:, :], osb[:, :], start=True, stop=True)

            if j == 0:
                ysbs[g] = y_pool.tile([C, G, N], BF, tag="ysb", name=f"ysb{g}")
            nc.vector.tensor_copy(ysbs[g][:, j, :], nd[:, 0:N])
            if j == G - 1:
                ysb = ysbs.pop(g)
                nc.sync.dma_start(yT[:, g * G:(g + 1) * G, :], ysb[:, :, :])

        # Software pipeline, steady-state iteration i:
        #   DVE: ebmul(i-2), recip/osb(i-3), qkt/vsb casts(i)
        #   ACT: ln/exp2(i-3), ysb-copy(i-3), exp(i)
        #   PE : qk/v(i), PV/den(i-2), proj(i-3), st(i)
        # Every op is >= 1 iteration downstream of its producers, so no
        # engine queue head ever blocks on same-iteration work.
        D2, D3 = PIPE, PIPE + 1
        for i in range(n_windows + D3):
            if 0 <= i - D2 < n_windows:
                ebmul(i - D2)
            if 0 <= i - D3 < n_windows:
                norm(i - D3)
            if i < n_windows:
                front_a(i)
            if 0 <= i - D2 < n_windows:
                backmm(i - D2)
            if 0 <= i - D3 < n_windows:
                fin(i - D3)
            if i < n_windows:
                front_b(i)

    _split_waits(nc)
    return nc


def _host_bias(pp_w, pp_b, ln1_g, ln1_b, l1_w, l1_b, ln2_g, ln2_b, l2_w, l2_b,
               ln3_g, ln3_b, l3_w, l3_b):
    """Replicates the reference's tiny position-bias MLP in numpy fp32."""
    p = np.arange(1 - GS, GS)
    bb = np.stack(np.meshgrid(p, p, indexing="ij")).reshape(2, -1).T.astype(np.float32)

    def ln(x, g, b):
        mu = x.mean(-1, keepdims=True)
        var = ((x - mu) ** 2).mean(-1, keepdims=True)
        return (x - mu) / np.sqrt(var + EPS) * g + b

    pos = bb @ pp_w + pp_b
    pos = np.maximum(ln(pos, ln1_g, ln1_b), 0) @ l1_w + l1_b
    pos = np.maximum(ln(pos, ln2_g, ln2_b), 0) @ l2_w + l2_b
    pos = np.maximum(ln(pos, ln3_g, ln3_b), 0) @ l3_w + l3_b   # [729, H]

    ch = np.arange(GS)
    coords = np.stack(np.meshgrid(ch, ch, indexing="ij")).reshape(2, -1)
    rel = coords[:, :, None] - coords[:, None, :]
    rel = rel.transpose(1, 2, 0) + (GS - 1)
    idx = rel[..., 0] * (2 * GS - 1) + rel[..., 1]               # [N, N]
    return pos[idx]                                              # [N, N, H] = bias[q,k,h]


_NC_CACHE = {}


def kernel(**inputs):
    x = np.asarray(inputs["x"], dtype=np.float32)
    scale = np.float32(HD) ** -0.5

    rpb = _host_bias(*[np.asarray(inputs[k], dtype=np.float32) for k in
                       ("pp_w", "pp_b", "ln1_g", "ln1_b", "l1_w", "l1_b",
                        "ln2_g", "ln2_b", "l2_w", "l2_b",
                        "ln3_g", "ln3_b", "l3_w", "l3_b")])
    # EB[r, (h, c, q)] = exp(bias[q, 98c+r, h]) matching ST tile layout
    ebt = np.exp(rpb.transpose(2, 1, 0))            # [H, k, q]
    ebm = np.empty((KC, H, 2, N), dtype=np.float32)
    for c in range(2):
        ebm[:, :, c, :] = ebt.transpose(1, 0, 2)[c * KC:(c + 1) * KC]
    ebm = ebm.reshape(KC, H, 2 * N)

    wkv = np.asarray(inputs["wkv"], dtype=np.float32)
    consts = {
        "eb": np.ascontiguousarray(ebm.astype(BF_NP)),
        "wq": np.ascontiguousarray((np.asarray(inputs["wq"], np.float32) * scale).astype(BF_NP)),
        "wk": np.ascontiguousarray(wkv[:, :C].astype(BF_NP)),
        "wv": np.ascontiguousarray(wkv[:, C:].astype(BF_NP)),
        "wproj": np.ascontiguousarray(np.asarray(inputs["wproj"], np.float32).astype(BF_NP)),
        "ones": np.ones((KC, HD), dtype=BF_NP),
    }
    bproj = np.asarray(inputs["bproj"], dtype=np.float32)

    # [B, N, C] -> per-core [C, W, N] bf16
    xt_all = x.transpose(2, 0, 1).astype(BF_NP)      # [C, B, N]

    if W not in _NC_CACHE:
        _NC_CACHE[W] = _build(W)
    nc = _NC_CACHE[W]

    in_maps = []
    for core in range(NCORES):
        m = dict(consts)
        m["xT"] = np.ascontiguousarray(xt_all[:, core * W:(core + 1) * W, :])
        in_maps.append(m)

    import os
    trace = bool(os.environ.get("BASS_KERNEL_TRACE"))
    res = run_bass_kernel_spmd(nc, in_maps, core_ids=list(range(NCORES)),
                               trace=trace)
    global LAST_RESULT
    LAST_RESULT = res

    out = np.empty((B, N, C), dtype=np.float32)
    for core in range(NCORES):
        yt = res.results[core]["yT"]                 # [C, W, N] bf16
        out[core * W:(core + 1) * W] = yt.astype(np.float32).transpose(1, 2, 0)
    out += bproj[None, None, :]
    return out


LAST_RESULT = None
